# revision 23
# baseline (speedup 1.0000x reference)
"""Multi-head causal attention (B=1, S=4096, D=768, H=12) on 8 trn2 NeuronCores.

Sharding: tensor-parallel over heads + causal-balanced split of the query range.
  - cores 0-5 ("late"):  2 heads each, q in [1536, 4096), k in [0, 4096)
  - cores 6-7 ("early"): 6 heads each, q in [0, 1536),  k in [0, 1536)

v2 design (vs the 195us chunk-wavefront baseline):
  - v-bias is folded into the host-side output bias: softmax rows sum to 1,
    so attn@(v + 1*bv^T) = attn@v0 + bv, and bv@W_out^T joins b_out on the
    host.  Kills every v-bias DVE op and the bvb input.
  - fine-grained projection pieces: k/q pieces per XW=512 block, v pieces
    per CK=256 chunk (~1-1.5us of PE each), interleaved between attention
    groups via a statically planned lookahead (k/q two groups ahead of the
    scores that need them; v pieces land between their group's scores and
    its AV flush).  The scalar engine's exp stream never sees a multi-us
    projection bubble.
  - lean head: weight DMA split per q/k/v slice and ordered so the first
    k-block piece starts ~2.5us in; first scores ~7us (was ~30us).
  - global AV stagger: group g's AV is emitted after group g+1's scores
    ACROSS pair and qtile boundaries, so the exp stream stays dense at the
    seams.  start=True only on the first matmul into each av bank,
    stop=True only on the last.
  - out-projection runs as 6 per-qtile pieces in the "big" psum tag
    rotation (transient, like scores/projection pieces); the old separate
    "av"-tag scheme made outproj wait on the NEXT qtile's normalize.
  - diag masking: one strided tensor_mul applies tri to (j0, q-first-half)
    and (j1, q-second-half); the j1 AV matmul only covers q-second-half so
    the untouched garbage-free exp of the masked block is never read.
  - normalize: dens -> sbuf row, gpsimd partition_broadcast to 128 rows,
    one reciprocal_approx_fast per (pair, qtile) (full-height base-0 tiles
    only - HW quirk), numer * recip on DVE.
  - y written as bf16 partials, one strided DMA per qtile issued from the
    gpsimd queue (keeps the sync queue x-block-only); host sums the core
    partials in fp32 and adds b_out + bv@W_out^T.

All inputs are taken at full shape; slicing/packing happens on host.
"""

import sys
import threading

sys.path.insert(0, "/opt/trn_rl_repo")

import numpy as np
import ml_dtypes

import concourse.bass as bass
import concourse.mybir as mybir
import concourse.tile as tile
from concourse import bacc

# ---------------------------------------------------------------- constants
B, S, D, H, DH = 1, 4096, 768, 12, 64
SCALE = DH ** -0.5
P = 128          # sbuf partitions
QT = 256         # query tile (free axis of scores)
KT = 128         # key tile (partition axis of scores)
CK = 256         # projection chunk (seq)
XW = 512         # x dma tile width (2 chunks)
GMAX = 2         # ktiles per score/exp group (psum: [P,2,GMAX,QT] = 4KB)
EXG = 3          # ex sbuf tiles stay 3 wide so the strided diag view works
SPLIT = 1536     # early/late query split point
DT = mybir.dt.float32
BF = mybir.dt.bfloat16

CLASSES = {
    # name: (n_pairs, q0, q1, k_len)
    "late": (1, SPLIT, S, S),
    "early": (3, 0, SPLIT, SPLIT),
}


def _groups(n):
    """Split n (even) non-diagonal ktiles into chunks of 3 and 2."""
    out = []
    while n >= 5 or n == 3:
        out.append(3)
        n -= 3
    while n > 0:
        out.append(2)
        n -= 2
    return out


def build_module(cls):
    n_pairs, q0, q1, k_len = CLASSES[cls]
    f_c = P * n_pairs            # per-core feature width of each projection
    q_len = q1 - q0
    n_ck = k_len // CK           # projection chunks
    n_blk = (k_len + XW - 1) // XW
    n_kt = k_len // KT           # ktiles of the core's k-support
    n_qt = q_len // QT           # qtiles of the core's q-range
    n_dt = D // P                # 6 contraction tiles for the projections
    qt0_g = q0 // QT             # global index of the core's first qtile

    nc = bacc.Bacc("TRN2", target_bir_lowering=False, debug=False,
                   enable_asserts=True, num_devices=1)

    xT = nc.dram_tensor("xT", [D, k_len], BF, kind="ExternalInput")
    wqkvT = nc.dram_tensor("wqkvT", [P, 3 * n_dt * f_c], BF,
                           kind="ExternalInput")
    bq = nc.dram_tensor("bq", [f_c, 1], DT, kind="ExternalInput")
    woT = nc.dram_tensor("woT", [P, n_pairs * n_dt * P], BF,
                         kind="ExternalInput")
    dmask = nc.dram_tensor("dmask", [P, 4 * KT], BF, kind="ExternalInput")
    yT = nc.dram_tensor("yT", [D, q_len], BF, kind="ExternalOutput")

    # ---------------- static plans -------------------------------------
    def plan_for(qt):
        n_kt_q = 2 * (qt0_g + qt) + 2
        return ([(2, False)] * ((n_kt_q - 2) // 2) + [(2, True)], n_kt_q)

    # group stream: (qt, p, gi, kts, diag, first, last, n_kt_q)
    stream = []
    for qt in range(n_qt):
        plan, n_kt_q = plan_for(qt)
        for p in range(n_pairs):
            kt0 = 0
            for gi, (gsz, diag) in enumerate(plan):
                kts = list(range(kt0, kt0 + gsz))
                kt0 += gsz
                stream.append((qt, p, gi, kts, diag, gi == 0,
                               gi == len(plan) - 1, n_kt_q))
    n_groups = len(stream)

    # per-group piece requirements, all single-chunk (CK) granularity.
    # reqs[t] is pulled when group t-LA is emitted (LA=3, so a k/q piece's
    # DVE evac has ~3 groups to land before the scores read it); v pieces
    # attach two groups late so they land between their group's scores and
    # its AV flush (which runs after group+1's scores).
    LA = 3
    reqs = [[] for _ in range(n_groups + LA + 1)]
    for t, (qt, p, gi, kts, diag, first, last, n_kt_q) in enumerate(stream):
        for k in kts:
            reqs[t].append(("k", (k * KT) // CK, p))
        if first:
            reqs[t].append(("q", (q0 + qt * QT) // CK, p))
        for k in kts:
            reqs[t + 2].append(("v", (k * KT) // CK, p))

    # dedupe into the actual emission sequence: head = reqs[0..LA-1],
    # then group t pulls reqs[t+LA]
    piece_seq = []
    seen = set()

    def _pull(lst, sink):
        for key in lst:
            if key not in seen:
                seen.add(key)
                sink.append(key)

    head_pieces = []
    for t in range(LA):
        _pull(reqs[t], head_pieces)
    pulls = [[] for _ in range(n_groups)]
    for t in range(n_groups):
        _pull(reqs[t + LA], pulls[t])
    piece_seq = head_pieces + [k for pl in pulls for k in pl]
    # safety: every projection piece must be demanded by some group
    assert len(piece_seq) == (2 * n_ck + n_qt) * n_pairs

    # x-block DMA issue schedule: block b issues 2 piece-slots before its
    # first consumer (min 0 = preamble), in first-use order
    first_use = {}
    for i, (kind, idx, p) in enumerate(piece_seq):
        b = (idx * CK) // XW
        first_use.setdefault(b, i)
    issue_order = sorted(first_use, key=lambda b: first_use[b])
    issue_at = {}   # piece index -> list of blocks to issue just before
    preamble_blocks = []
    for b in issue_order:
        pos = first_use[b] - 2
        if pos <= 0 or first_use[b] < len(head_pieces):
            preamble_blocks.append(b)
        else:
            issue_at.setdefault(pos, []).append(b)

    with tile.TileContext(nc) as tc:
        with (
            tc.tile_pool(name="w", bufs=1) as sb_w,
            tc.tile_pool(name="x", bufs=5) as sb_x,
            tc.tile_pool(name="persist", bufs=1) as sb_per,
            tc.tile_pool(name="exp", bufs=5) as sb_exp,
            tc.tile_pool(name="aTp", bufs=2) as sb_a,
            tc.tile_pool(name="rn", bufs=3) as sb_rn,
            tc.tile_pool(name="yout", bufs=2) as sb_y,
            tc.tile_pool(name="big", bufs=3, space="PSUM") as ps_big,
            tc.tile_pool(name="av", bufs=2, space="PSUM") as ps_av,
        ):
            # ---------------- weights: split DMAs, lean ordering
            wqkv_sb = sb_w.tile([P, 3, n_dt, f_c], BF, tag="wqkv")
            wqkv_r = wqkvT.rearrange("p (g t f) -> p g t f", g=3, t=n_dt)
            # k-slice first (first k piece needs it), then the first x block
            nc.sync.dma_start(out=wqkv_sb[:, 1, :, :], in_=wqkv_r[:, 1, :, :])

            blocks = {}

            def issue_block(b):
                if b in blocks or b >= n_blk:
                    return
                ps0 = b * XW
                w = min(XW, k_len - ps0)
                xt = sb_x.tile([P, n_dt, XW], BF, tag="xt", name="xt")
                nc.sync.dma_start(
                    out=xt[:, :, :w],
                    in_=xT.rearrange("(t p) s -> p t s", p=P)[
                        :, :, ps0:ps0 + w])
                blocks[b] = xt

            # x blocks feeding the first scores go out before the non-k
            # weights: the DMA engines drain in issue order and the first
            # exp is gated on them
            issue_block(preamble_blocks[0])
            if len(preamble_blocks) > 1:
                issue_block(preamble_blocks[1])
            nc.sync.dma_start(out=wqkv_sb[:, 0, :, :], in_=wqkv_r[:, 0, :, :])
            bq_sb = sb_w.tile([P, n_pairs], DT, tag="bq")
            nc.sync.dma_start(
                out=bq_sb, in_=bq.rearrange("(n p) o -> p (n o)", p=P))
            for b in preamble_blocks[2:]:
                issue_block(b)
            dmask_sb = sb_w.tile([P, 2, 2, KT], BF, tag="dmask")
            nc.sync.dma_start(
                out=dmask_sb,
                in_=dmask.rearrange("p (h b c) -> p h b c", h=2, b=2))
            nc.sync.dma_start(out=wqkv_sb[:, 2, :, :], in_=wqkv_r[:, 2, :, :])
            wo_sb = sb_w.tile([P, n_pairs, n_dt, P], BF, tag="wo")
            nc.sync.dma_start(
                out=wo_sb,
                in_=woT.rearrange("p (n t m) -> p n t m", n=n_pairs, t=n_dt))

            # ---------------- PE warmup: dummy matmuls during the initial
            # DMA wait so the HAM clock-gate reaches 8/8 before real work
            wup = sb_w.tile([P, QT], BF, tag="wup")
            nc.vector.memset(wup, 0.0)
            for _ in range(20):
                wps = ps_av.tile([P, 2, QT], DT, tag="av", name="wps")
                nc.tensor.matmul(wps[:, 0, :], wup[:, 0:128], wup,
                                 start=True, stop=True)
                nc.tensor.matmul(wps[:, 1, :], wup[:, 0:128], wup,
                                 start=True, stop=True)

            # ---------------- persistent activations (head pair packed on
            # partitions: head A rows 0-63, head B rows 64-127)
            qT = [sb_per.tile([P, q_len], BF, tag=f"qT{p}", name=f"qT{p}")
                  for p in range(n_pairs)]
            kT = [sb_per.tile([P, k_len], BF, tag=f"kT{p}", name=f"kT{p}")
                  for p in range(n_pairs)]
            # per ktile: [V_A(64) | 1 | pad | V_B(64) | 1 | pad], k on parts
            vkt = [sb_per.tile([P, n_kt, 132], BF, tag=f"vk{p}", name=f"vk{p}")
                   for p in range(n_pairs)]
            for p in range(n_pairs):
                nc.vector.memset(vkt[p][:, :, 64:65], 1.0)
                nc.vector.memset(vkt[p][:, :, 130:131], 1.0)

            # ---------------- projection pieces ------------------------
            def emit_piece(kind, idx, p):
                if kind in ("k", "q"):
                    s0 = idx * CK
                    b = s0 // XW
                    co = s0 - b * XW
                    xt = blocks[b]
                    gsl = 1 if kind == "k" else 0
                    ps = ps_big.tile([P, CK], DT, tag="big", name="ps_kq")
                    for dti in range(n_dt):
                        nc.tensor.matmul(
                            ps,
                            wqkv_sb[:, gsl, dti, p * P:(p + 1) * P],
                            xt[:, dti, co:co + CK],
                            start=dti == 0, stop=dti == n_dt - 1)
                    if kind == "k":
                        if n_pairs > 1:
                            nc.scalar.copy(kT[p][:, s0:s0 + CK], ps)
                        else:
                            nc.vector.tensor_copy(kT[p][:, s0:s0 + CK], ps)
                    else:
                        lo = max(s0, q0)
                        nc.vector.tensor_scalar_add(
                            qT[p][:, lo - q0:s0 + CK - q0],
                            ps[:, lo - s0:CK], bq_sb[:, p:p + 1])
                else:  # v piece, one CK chunk, [seq, dh] layout directly
                    c = idx
                    s0 = c * CK
                    b = s0 // XW
                    xt = blocks[b]
                    co = s0 - b * XW
                    ps = ps_big.tile([P, 2, KT], DT, tag="big", name="ps_v")
                    for j in range(2):
                        so = co + j * KT
                        for dti in range(n_dt):
                            nc.tensor.matmul(
                                ps[:, j, :],
                                xt[:, dti, so:so + KT],
                                wqkv_sb[:, 2, dti, p * P:(p + 1) * P],
                                start=dti == 0, stop=dti == n_dt - 1)
                    kt_i = s0 // KT
                    dst = vkt[p][:, kt_i:kt_i + 2, :].rearrange(
                        "p k (h c) -> p k h c", h=2)[:, :, :, 0:64]
                    nc.vector.tensor_copy(
                        dst, ps.rearrange("p j (h c) -> p j h c", h=2))

            piece_pos = 0

            def emit_pieces(lst):
                nonlocal piece_pos
                for key in lst:
                    for b in issue_at.get(piece_pos, ()):
                        issue_block(b)
                    emit_piece(*key)
                    piece_pos += 1

            # ---------------- attention stream -------------------------
            avs = {}       # (qt, p) -> av psum tile
            a_tiles = {}   # (qt, p) -> normalized aT sbuf tile
            pend = None    # (qt, p, kts, ex, diag, first, last, n_kt_q)
            post_q = []    # [min_group, item] deferred outproj / ydma
            ysbs = {}
            cur_t = [0]    # current group index (read by flush_pend)

            def flush_pend():
                nonlocal pend
                if pend is None:
                    return
                qt, p, kts, ex, diag, first, last, n_kt_q = pend
                pend = None
                av = avs[(qt, p)]
                mms = []  # (out, stationary, moving)
                if diag and first:
                    # single-group qtile (qt'==0): emit_group zeroed the
                    # masked (j1, q-first-half) ex, so j1 runs full width and
                    # every av byte is touched while uniformly pending
                    diag = False
                if diag:
                    # diag groups are exactly 2 ktiles.  j1 covers only the
                    # q-second-half (its first half is fully masked and the
                    # exp there was never zeroed); j0 full-width goes last.
                    k0, k1 = kts
                    h = QT // 2
                    for hi in (0, 1):
                        mms.append((av[0:65, hi, h:],
                                    vkt[p][:, k1, 66 * hi:66 * hi + 65],
                                    ex[:, hi, 1, h:]))
                    for hi in (0, 1):
                        mms.append((av[0:65, hi, :],
                                    vkt[p][:, k0, 66 * hi:66 * hi + 65],
                                    ex[:, hi, 0, :]))
                else:
                    for j, k in enumerate(kts):
                        for hi in (0, 1):
                            mms.append((av[0:65, hi, :],
                                        vkt[p][:, k, 66 * hi:66 * hi + 65],
                                        ex[:, hi, j, :]))
                for i, (o, v, e) in enumerate(mms):
                    nc.tensor.matmul(o, v, e,
                                     start=(first and i == 0),
                                     stop=(last and i == len(mms) - 1))
                if last:
                    # normalize latency chain inline (DVE/gpsimd-only, no PE
                    # content); the muls defer +2 groups so the boundary DVE
                    # queue stays short, and the out-projection matmuls +3
                    # groups so the PE queue never head-of-line blocks on
                    # the chain (which also drops the PE pstate)
                    emit_norm_a(qt, p)
                    post_q.append([cur_t[0] + 2, ("normb", qt, p)])
                    if p == n_pairs - 1:
                        for mt in range(n_dt):
                            post_q.append([cur_t[0] + 3, ("oproj", qt, mt)])
                        post_q.append([cur_t[0] + 3, ("ydma", qt)])

            def emit_group(t):
                qt, p, gi, kts, diag, first, last, n_kt_q = stream[t]
                if first:
                    avs[(qt, p)] = ps_av.tile([P, 2, QT], DT, tag="av",
                                              name="av")
                qh = [qT[p][hi * 64:(hi + 1) * 64,
                            qt * QT:(qt + 1) * QT] for hi in (0, 1)]
                ps_sc = ps_big.tile([P, 2, GMAX, QT], DT, tag="big",
                                    name="ps_sc")
                for j, k in enumerate(kts):
                    for hi in (0, 1):
                        nc.tensor.matmul(
                            ps_sc[:, hi, j, :],
                            kT[p][hi * 64:(hi + 1) * 64,
                                  k * KT:(k + 1) * KT],
                            qh[hi], start=True, stop=True)
                gsz = len(kts)
                ex = sb_exp.tile([P, 2, EXG, QT], BF, tag="ex")
                nc.scalar.activation(
                    ex[:, :, 0:gsz, :], ps_sc[:, :, 0:gsz, :],
                    mybir.ActivationFunctionType.Exp, scale=SCALE)
                if diag:
                    # tri on (j0, q-first-half) and (j1, q-second-half):
                    # offsets hi*768 + jj*384, one strided view
                    exv = ex.rearrange("p h g q -> p h (g q)").rearrange(
                        "p h (b a c) -> p h b a c", b=2, c=KT)[:, :, :, 0, :]
                    nc.vector.tensor_mul(exv, exv, dmask_sb)
                    if first:
                        # single-group qtile: j1 AV runs full width, so the
                        # fully-masked (j1, q-first-half) must be zeroed
                        nc.vector.memset(ex[:, :, 1, 0:KT], 0.0)
                return (qt, p, kts, ex, diag, first, last, n_kt_q)

            rbs = {}

            def emit_norm_a(qt, p):
                av = avs[(qt, p)]
                dd = sb_rn.tile([1, 2 * QT], DT, tag="dd")
                if n_pairs > 1:
                    nc.scalar.copy(dd, av[64:65, :, :].rearrange(
                        "p h q -> p (h q)"))
                else:
                    nc.vector.tensor_copy(dd, av[64:65, :, :].rearrange(
                        "p h q -> p (h q)"))
                db = sb_rn.tile([P, 2 * QT], DT, tag="db")
                nc.gpsimd.partition_broadcast(db, dd)
                rb = sb_rn.tile([P, 2, QT], DT, tag="rb")
                nc.vector.reciprocal_approx_fast(
                    rb.rearrange("p h q -> p (h q)"), db)
                rbs[(qt, p)] = rb

            def emit_post(item):
                kind = item[0]
                if kind == "normb":
                    _, qt, p = item
                    av = avs[(qt, p)]
                    rb = rbs.pop((qt, p))
                    aT = sb_a.tile([P, QT], BF, tag=f"aT{p}", name="aT")
                    for hi in (0, 1):
                        nc.vector.tensor_mul(
                            aT[hi * 64:(hi + 1) * 64, :],
                            av[0:64, hi, :], rb[hi * 64:hi * 64 + 64, hi, :])
                    a_tiles[(qt, p)] = aT
                elif kind == "oproj":
                    _, qt, mt = item
                    if mt == 0:
                        ysbs[qt] = sb_y.tile([P, n_dt, QT], BF, tag="y",
                                             name="ysb")
                    ps_y = ps_big.tile([P, QT], DT, tag="big", name="ps_y")
                    for p in range(n_pairs):
                        nc.tensor.matmul(
                            ps_y, wo_sb[:, p, mt, :], a_tiles[(qt, p)],
                            start=(p == 0), stop=(p == n_pairs - 1))
                    nc.vector.tensor_copy(ysbs[qt][:, mt, :], ps_y)
                else:  # ydma
                    _, qt = item
                    nc.gpsimd.dma_start(
                        out=yT.rearrange("(t p) q -> p t q", p=P)[
                            :, :, qt * QT:(qt + 1) * QT],
                        in_=ysbs[qt])

            # ---------------- main loop --------------------------------
            emit_pieces(head_pieces)
            for t in range(n_groups):
                cur_t[0] = t
                new_pend = emit_group(t)
                flush_pend()
                pend = new_pend
                emit_pieces(pulls[t])
                if post_q and post_q[0][0] <= t:
                    emit_post(post_q.pop(0)[1])
            cur_t[0] = n_groups
            flush_pend()
            while post_q:
                emit_post(post_q.pop(0)[1])
            assert len(seen) == len(piece_seq) and len(blocks) == n_blk

    nc.compile()
    return nc


# ---------------------------------------------------------------- host side
def _head_cols(heads):
    """column indices into a [*, 768] head-blocked axis for the given heads"""
    return np.concatenate([np.arange(h * DH, (h + 1) * DH) for h in heads])


def make_in_maps(x, W_in, b_in, W_out):
    """Returns (late_in_maps[6], early_in_maps[2])."""
    xT = np.ascontiguousarray(x.reshape(S, D).T).astype(ml_dtypes.bfloat16)
    WT = np.ascontiguousarray(W_in.T)                     # [768, 2304]
    WoT = np.ascontiguousarray(W_out.T)                   # [768, 768]

    tri = np.triu(np.ones((P, P), np.float32))            # k <= q
    dm = np.concatenate([tri, tri, tri, tri], axis=1)     # [h, b] x tri
    dm = dm.astype(ml_dtypes.bfloat16)

    def core_inputs(heads, cls):
        _, q0, q1, k_len = CLASSES[cls]
        cols = _head_cols(heads)
        bf = ml_dtypes.bfloat16
        wq = np.ascontiguousarray(WT[:, cols])
        wk = np.ascontiguousarray(WT[:, 768 + cols])
        wv = np.ascontiguousarray(WT[:, 1536 + cols])
        f_cc = len(cols)
        wqkv = np.concatenate([wq, wk, wv], axis=1)      # [768, 3*f_c]
        wqkv = (wqkv.reshape(6, 128, 3, f_cc).transpose(1, 2, 0, 3)
                .reshape(128, 18 * f_cc)).astype(bf)     # [p, (g, t, f)]
        bqc = np.ascontiguousarray(b_in[cols][:, None]).astype(np.float32)
        wo = WoT[cols, :]                                # [f_c, 768]
        wo = (wo.reshape(f_cc // 128, 128, 6, 128).transpose(1, 0, 2, 3)
              .reshape(128, -1)).astype(bf)
        return {
            "xT": np.ascontiguousarray(xT[:, :k_len]),
            "wqkvT": np.ascontiguousarray(wqkv),
            "bq": bqc, "woT": wo, "dmask": dm,
        }

    late = [core_inputs([2 * c, 2 * c + 1], "late") for c in range(6)]
    early = [core_inputs(list(range(6 * e, 6 * e + 6)), "early")
             for e in range(2)]
    return late, early


def assemble_output(late_res, early_res, b_in, b_out, W_out):
    yT = np.zeros((D, S), np.float32)
    for r in late_res:
        yT[:, SPLIT:] += np.asarray(r["yT"], dtype=np.float32)
    for r in early_res:
        yT[:, :SPLIT] += np.asarray(r["yT"], dtype=np.float32)
    # v-bias is exact through softmax (rows sum to 1): fold bv@W_out^T here
    b_eff = b_out + np.asarray(b_in[1536:], np.float32) @ np.asarray(
        W_out, np.float32).T
    y = yT.T + b_eff[None, :]
    return y.reshape(B, S, D).astype(np.float32)


# ------------------------------------------- pjrt runner (explicit devices)
def _run_group(nc, in_maps, devices):
    """run_bass_via_pjrt equivalent on an explicit device subset."""
    import jax
    from jax.sharding import Mesh, PartitionSpec
    from jax.experimental.shard_map import shard_map
    from concourse import bass2jax
    from concourse.bass2jax import _bass_exec_p, partition_id_tensor

    bass2jax.install_neuronx_cc_hook()
    n_cores = len(in_maps)
    partition_name = (nc.partition_id_tensor.name
                      if nc.partition_id_tensor else None)

    in_names, out_names, out_avals, zero_outs = [], [], [], []
    for alloc in nc.m.functions[0].allocations:
        if not isinstance(alloc, mybir.MemoryLocationSet):
            continue
        name = alloc.memorylocations[0].name
        if alloc.kind == "ExternalInput":
            if name != partition_name:
                in_names.append(name)
        elif alloc.kind == "ExternalOutput":
            shape = tuple(alloc.tensor_shape)
            dtype = mybir.dt.np(alloc.dtype)
            out_names.append(name)
            out_avals.append(jax.core.ShapedArray(shape, dtype))
            zero_outs.append(np.zeros(shape, dtype))
    n_params = len(in_names)
    n_outs = len(out_avals)
    in_names = in_names + out_names
    if partition_name is not None:
        in_names.append(partition_name)
    donate = tuple(range(n_params, n_params + n_outs))

    def _body(*args):
        operands = list(args)
        if partition_name is not None:
            operands.append(partition_id_tensor())
        outs = _bass_exec_p.bind(
            *operands,
            out_avals=tuple(out_avals),
            in_names=tuple(in_names),
            out_names=tuple(out_names),
            lowering_input_output_aliases=(),
            sim_require_finite=True,
            sim_require_nnan=True,
            nc=nc,
        )
        return tuple(outs)

    per_core = [[np.asarray(m[name]) for name in in_names[:n_params]]
                for m in in_maps]
    if n_cores == 1:
        out_arrs = jax.jit(_body, donate_argnums=donate, keep_unused=True)(
            *per_core[0], *zero_outs)
        return [{n: np.asarray(out_arrs[i]) for i, n in enumerate(out_names)}]

    mesh = Mesh(np.asarray(devices), ("core",))
    in_specs = (PartitionSpec("core"),) * (n_params + n_outs)
    out_specs = (PartitionSpec("core"),) * len(out_names)
    sharded = jax.jit(
        shard_map(_body, mesh=mesh, in_specs=in_specs, out_specs=out_specs,
                  check_rep=False),
        donate_argnums=donate, keep_unused=True)
    concat_in = [np.concatenate([per_core[c][i] for c in range(n_cores)],
                                axis=0) for i in range(n_params)]
    concat_zeros = [np.zeros((n_cores * z.shape[0], *z.shape[1:]), z.dtype)
                    for z in zero_outs]
    out_arrs = sharded(*concat_in, *concat_zeros)
    return [
        {n: np.asarray(out_arrs[i]).reshape(n_cores, *out_avals[i].shape)[c]
         for i, n in enumerate(out_names)}
        for c in range(n_cores)
    ]


_MODULES = {}
_WARM = set()


def _get_module(cls):
    if cls not in _MODULES:
        _MODULES[cls] = build_module(cls)
    return _MODULES[cls]


def kernel(x, W_in, b_in, W_out, b_out):
    import jax
    x = np.asarray(x, np.float32)
    W_in = np.asarray(W_in, np.float32)
    b_in = np.asarray(b_in, np.float32)
    W_out = np.asarray(W_out, np.float32)
    b_out = np.asarray(b_out, np.float32)

    late_maps, early_maps = make_in_maps(x, W_in, b_in, W_out)
    nc_late = _get_module("late")
    nc_early = _get_module("early")

    devs = jax.devices()
    results = {}
    errs = {}

    def run(tag, nc, maps, devices):
        try:
            results[tag] = _run_group(nc, maps, devices)
        except Exception as e:  # noqa: BLE001
            errs[tag] = e

    # first call per module compiles (serialize those); afterwards the two
    # device groups (cores 0-5 and 6-7) execute concurrently
    t1 = threading.Thread(target=run,
                          args=("late", nc_late, late_maps, devs[0:6]))
    t2 = threading.Thread(target=run,
                          args=("early", nc_early, early_maps, devs[6:8]))
    if not _WARM:
        t1.start(); t1.join()
        t2.start(); t2.join()
        _WARM.add(True)
    else:
        t1.start(); t2.start()
        t1.join(); t2.join()
    if errs:
        raise next(iter(errs.values()))

    return assemble_output(results["late"], results["early"], b_in, b_out,
                           W_out)
